# revision 1
# baseline (speedup 1.0000x reference)
"""MixtureOfDepth transformer on 8 trn2 NeuronCores (Bass/Tile).

DP-4 over batch x TP-2 within core pairs. x lives in DRAM between layers
(transposed [D, T]). 2 pairwise AllReduces per layer, each split in two
row-chunks so reduction overlaps the producing matmuls. LayerNorm residual is
computed in place in SBUF (no DRAM round-trip) and its output feeds the FFN
directly as SBUF tiles. Exact comparison-count top-k ranks; selected token
gather/scatter via indirect DMA on DRAM natural-layout staging.

Per-layer dtype via KMODES (default ffffff): 'f' = fp32 (exact — required for
layers feeding later top-k routing; boundary gaps are as small as 1.8e-5),
'r' = fp32r (4x faster PE, ~12 mantissa bits — safe only after the last
routing decision). The MoD routing path itself is always fp32.
"""
import os, sys
import numpy as np

sys.path.insert(0, "/opt/trn_rl_repo")
import concourse.bass as bass
import concourse.tile as tile
from concourse import bacc, mybir
from concourse import bass_utils
from contextlib import ExitStack, nullcontext

FP = mybir.dt.float32
FPR = mybir.dt.float32r
I32 = mybir.dt.int32


def R(ap):
    # tiles already carry the layer's dtype; kept as identity shim
    return ap


def mkdv(mode):
    """Per-layer dtype mode: 'r' -> fp32r tiles + bitcast DRAM views."""
    if mode == "r":
        return FPR, (lambda ap: ap.bitcast(FPR))
    return FP, (lambda ap: ap)
D, H, HD, DFF, NL, T, B = 1024, 16, 64, 4096, 6, 2048, 4
EPS = 1e-5
HH, DFH, KSEL = H // 2, 4096 // 2, T // 2
AF = mybir.ActivationFunctionType
OP = mybir.AluOpType
RG = [[0, 1], [2, 3], [4, 5], [6, 7]]

_CACHED = {}


class Ctr:
    def __init__(self):
        self.i = 0

    def nm(self, p):
        self.i += 1
        return f"{p}{self.i}"


def load_x(nc, pool, u, xd, Tl, tag="xin", DT=FP, V=None):
    if V is None:
        V = lambda ap: ap
    ts = []
    for dc in range(8):
        t = pool.tile([128, Tl], DT, tag=f"{tag}{dc}", bufs=1, name=u.nm(tag))
        nc.sync.dma_start(t[:], V(xd[128 * dc:128 * (dc + 1), :]))
        ts.append(t)
    return ts


def emit_ln(nc, tc, u, x_tiles, add_dram, g_col, b_col, C, Tl, out_dram, dram,
            DT=FP, V=None, out_tiles=None):
    """out <- LN(x + add). x_tiles: 8x [128,Tl] SBUF; residual add is done in
    place (x tiles are treated as dead). Output goes to out_tiles (SBUF) if
    given, else DMA'd to out_dram."""
    if V is None:
        V = lambda ap: ap
    sfx = "_r" if DT is FPR else ""
    ones_col = C["ones_col" + sfx]
    ones_row = C["ones_row" + sfx]
    NT = Tl // 512
    es = ExitStack()
    sb = es.enter_context(tc.tile_pool(name=u.nm("lnsb"), bufs=2))
    row = es.enter_context(tc.tile_pool(name=u.nm("lnrow"), bufs=6))
    esPA = ExitStack()
    psA = esPA.enter_context(tc.tile_pool(name=u.nm("lnpsA"), bufs=1, space="PSUM"))

    def rtile(nm):
        return row.tile([1, 512], DT, tag="rows", bufs=6, name=u.nm(nm))

    a1 = [psA.tile([1, 512], FP, tag=f"r1_{tb}", bufs=1, name=u.nm("r1"))
          for tb in range(NT)]
    a2 = [psA.tile([1, 512], FP, tag=f"r2_{tb}", bufs=1, name=u.nm("r2"))
          for tb in range(NT)]
    xrt = x_tiles  # residual add done in place; x tiles are dead afterwards
    for dc in range(8):
        t = x_tiles[dc]
        a = sb.tile([128, Tl], DT, tag="lnadd", bufs=1, name=u.nm("a"))
        nc.sync.dma_start(a[:], V(add_dram[128 * dc:128 * (dc + 1), :]))
        nc.vector.tensor_tensor(t[:], t[:], a[:], op=OP.add)
        x2 = sb.tile([128, Tl], DT, tag="x2", bufs=1, name=u.nm("x2"))
        nc.scalar.square(x2[:], t[:])
        for tb in range(NT):
            sl = slice(512 * tb, 512 * (tb + 1))
            nc.tensor.matmul(a1[tb][:], ones_col[:, 0:1], t[:, sl],
                             start=(dc == 0), stop=(dc == 7))
            nc.tensor.matmul(a2[tb][:], ones_col[:, 0:1], x2[:, sl],
                             start=(dc == 0), stop=(dc == 7))
    sxq = row.tile([1, 2 * Tl], DT, tag="sxq", bufs=1, name=u.nm("sxq"))
    sx = sxq[0:1, 0:Tl]
    sq = sxq[0:1, Tl:2 * Tl]
    for tb in range(NT):
        sl = slice(512 * tb, 512 * (tb + 1))
        nc.vector.tensor_copy(sx[0:1, sl], a1[tb][:])
        nc.vector.tensor_copy(sq[0:1, sl], a2[tb][:])
    esPA.close()
    psB = es.enter_context(tc.tile_pool(name=u.nm("lnpsB"), bufs=1, space="PSUM"))
    for tb in range(NT):
        sl = slice(512 * tb, 512 * (tb + 1))
        mu = rtile("mu")
        nc.vector.tensor_scalar(mu[:], sx[0:1, sl], 1.0 / D, None, OP.mult)
        veps = rtile("veps")
        nc.vector.tensor_scalar(veps[:], sq[0:1, sl], 1.0 / D, None, OP.mult)
        mu2 = rtile("mu2")
        nc.vector.tensor_tensor(mu2[:], mu[:], mu[:], op=OP.mult)
        veps2 = rtile("veps2")
        nc.vector.tensor_tensor(veps2[:], veps[:], mu2[:], op=OP.subtract)
        nc.vector.tensor_scalar(veps2[:], veps2[:], EPS, None, OP.add)
        s0 = rtile("s0")
        nc.scalar.sqrt(s0[:], veps2[:])
        r0 = rtile("r0")
        nc.vector.reciprocal(r0[:], s0[:])
        t1 = rtile("t1")
        nc.vector.tensor_tensor(t1[:], r0[:], r0[:], op=OP.mult)
        nc.vector.tensor_tensor(t1[:], t1[:], veps2[:], op=OP.mult)
        nc.vector.tensor_scalar(t1[:], t1[:], -0.5, 1.5, OP.mult, OP.add)
        rs = rtile("rs")
        nc.vector.tensor_tensor(rs[:], r0[:], t1[:], op=OP.mult)
        nmrs = rtile("nmrs")
        nc.vector.tensor_tensor(nmrs[:], mu[:], rs[:], op=OP.mult)
        nc.vector.tensor_scalar(nmrs[:], nmrs[:], -1.0, None, OP.mult)
        b1p = psB.tile([128, 512], FP, tag="bc1", bufs=2, name=u.nm("b1p"))
        nc.tensor.matmul(b1p[:], ones_row[0:1, 0:128], rs[0:1, :],
                         start=True, stop=True)
        b1s = sb.tile([128, 512], DT, tag="bc1s", bufs=2, name=u.nm("b1s"))
        nc.vector.tensor_copy(b1s[:], b1p[:])
        b2p = psB.tile([128, 512], FP, tag="bc2", bufs=2, name=u.nm("b2p"))
        nc.tensor.matmul(b2p[:], ones_row[0:1, 0:128], nmrs[0:1, :],
                         start=True, stop=True)
        b2s = sb.tile([128, 512], DT, tag="bc2s", bufs=2, name=u.nm("b2s"))
        nc.vector.tensor_copy(b2s[:], b2p[:])
        for dc in range(8):
            v1 = sb.tile([128, 512], DT, tag="v1", bufs=2, name=u.nm("v1"))
            nc.vector.tensor_tensor(v1[:], xrt[dc][:, sl], b1s[:], op=OP.mult)
            nc.vector.tensor_tensor(v1[:], v1[:], b2s[:], op=OP.add)
            if out_tiles is not None:
                nc.scalar.activation(out_tiles[dc][:, sl], v1[:], AF.Identity,
                                     bias=b_col[:, dc:dc + 1],
                                     scale=g_col[:, dc:dc + 1])
            else:
                o1 = sb.tile([128, 512], DT, tag="o1", bufs=2, name=u.nm("o1"))
                nc.scalar.activation(o1[:], v1[:], AF.Identity,
                                     bias=b_col[:, dc:dc + 1],
                                     scale=g_col[:, dc:dc + 1])
                nc.sync.dma_start(V(out_dram[128 * dc:128 * (dc + 1), sl]),
                                  o1[:])
    es.close()


def emit_encoder(nc, tc, u, li, Tl, x_dram, W, C, dram, out_dram, DT=FP, V=None):
    """Encoder layer reading x from DRAM [D, Tl], writing new x to out_dram."""
    if V is None:
        V = lambda ap: ap
    sfx = "_r" if DT is FPR else ""
    ones_row = C["ones_row" + sfx]
    NT = Tl // 512
    NTC = Tl // 128
    ar1 = dram.tile([D, Tl], FP, name=u.nm("ar1i"))
    ar1o = dram.tile([D, Tl], FP, name=u.nm("ar1o"))
    esXA = ExitStack()
    xap = esXA.enter_context(tc.tile_pool(name=u.nm("fxa"), bufs=1))
    esA = ExitStack()
    xp = esA.enter_context(tc.tile_pool(name=u.nm("axin"), bufs=1))
    x_tiles = load_x(nc, xp, u, x_dram, Tl, DT=DT, V=V)
    esW = ExitStack()
    sb = esW.enter_context(tc.tile_pool(name=u.nm("asb"), bufs=2))
    wsb = esW.enter_context(tc.tile_pool(name=u.nm("aw"), bufs=2))
    bqr = wsb.tile([1, 1024], DT, tag="bqr", bufs=1, name=u.nm("bqr"))
    nc.sync.dma_start(bqr[:], V(W["bqkv_row"][li]))
    bvr = wsb.tile([1, 512], DT, tag="bvr", bufs=1, name=u.nm("bvr"))
    nc.sync.dma_start(bvr[:], V(W["bv_row"][li]))
    bor = wsb.tile([1, 1024], DT, tag="bor", bufs=1, name=u.nm("bor"))
    nc.sync.dma_start(bor[:], V(W["bo_row"][li]))
    # oTn borrows the xa pool's buffers (tags xa0-3); lifetimes are disjoint
    oTn = [xap.tile([128, Tl], DT, tag=f"xa{i}", bufs=1, name=u.nm("oT"))
           for i in range(4)]
    for g in range(4):  # 2-head groups
        esG = ExitStack()
        gp = esG.enter_context(tc.tile_pool(name=u.nm("gq"), bufs=1))
        ps = esG.enter_context(tc.tile_pool(name=u.nm("gps"), bufs=1, space="PSUM"))
        qT = gp.tile([128, Tl], DT, tag="qT", bufs=1, name=u.nm("qT"))
        kT = gp.tile([128, Tl], DT, tag="kT", bufs=1, name=u.nm("kT"))
        vA = [gp.tile([128, 130], DT, tag=f"vA{i % 4}", bufs=(NTC + 3) // 4,
                      name=u.nm("vA")) for i in range(NTC)]
        for role, dst in ((0, qT), (1, kT)):  # chunk: q=g, k=4+g
            cc = g if role == 0 else 4 + g
            wt = wsb.tile([128, 1024], DT, tag="wqkv", bufs=2, name=u.nm("wq"))
            nc.sync.dma_start(wt[:], V(W["wqkv_packed"][li, cc]))
            for tb in range(NT):
                sl = slice(512 * tb, 512 * (tb + 1))
                acc = ps.tile([128, 512], FP, tag="qacc", bufs=2, name=u.nm("qa"))
                for dc in range(8):
                    nc.tensor.matmul(acc[:], wt[:, 128 * dc:128 * (dc + 1)],
                                     x_tiles[dc][:, sl], start=(dc == 0),
                                     stop=False)
                nc.tensor.matmul(acc[:], bqr[0:1, 128 * cc:128 * (cc + 1)],
                                 ones_row[0:1, 0:512], start=False,
                                 stop=True)
                nc.vector.tensor_copy(dst[:, sl], acc[:])
        vs = slice(128 * g, 128 * (g + 1))
        wvg = []
        for dc in range(8):
            wt = gp.tile([128, 128], DT, tag=f"wv{dc}", bufs=1, name=u.nm("wv"))
            nc.sync.dma_start(wt[:], V(W["wv_rows"][li, dc][:, vs]))
            wvg.append(wt)
        for ti in range(NTC):
            acc = ps.tile([128, 128], FP, tag="vacc", bufs=1, name=u.nm("va"))
            for dc in range(8):
                nc.tensor.matmul(acc[:],
                                 x_tiles[dc][:, 128 * ti:128 * (ti + 1)],
                                 wvg[dc][:, :], start=(dc == 0), stop=False)
            nc.tensor.matmul(acc[:], ones_row[0:1, 0:128], bvr[0:1, vs],
                             start=False, stop=True)
            ones_c = C["ones_col" + sfx]
            nc.vector.tensor_copy(vA[ti][:, 64:65], ones_c[:, 0:1])
            nc.vector.tensor_copy(vA[ti][:, 129:130], ones_c[:, 0:1])
            src = acc[:, :].rearrange("p (h c) -> p h c", c=64)
            dst = vA[ti][:, :].rearrange("p (h c) -> p h c", c=65)[:, :, 0:64]
            nc.vector.tensor_copy(dst, src)
        for qb in range(NT):
            sl = slice(512 * qb, 512 * (qb + 1))
            oacc2 = [ps.tile([128, 512], FP, tag="oacc", bufs=2, name=u.nm("oa"))
                     for _ in range(2)]
            for kc in range(NTC):
                # back-to-back K=64 score matmuls on disjoint PE row groups
                # (partitions 0-63 / 64-127) -> they execute concurrently
                sp2 = []
                for hh in range(2):
                    hs = slice(64 * hh, 64 * hh + 64)
                    sp = ps.tile([128, 512], FP, tag="sT", bufs=2,
                                 name=u.nm("sT"))
                    nc.tensor.matmul(sp[:], kT[hs, 128 * kc:128 * (kc + 1)],
                                     qT[hs, sl], start=True, stop=True)
                    sp2.append(sp)
                for hh in range(2):
                    pT = sb.tile([128, 512], DT, tag="pT", bufs=2,
                                 name=u.nm("pT"))
                    nc.scalar.activation(pT[:], sp2[hh][:], AF.Exp, scale=0.125)
                    nc.tensor.matmul(oacc2[hh][0:65, :],
                                     vA[kc][:, 65 * hh:65 * hh + 65],
                                     pT[:], start=(kc == 0),
                                     stop=(kc == NTC - 1))
            for hh in range(2):
                hs = slice(64 * hh, 64 * hh + 64)
                rse = sb.tile([1, 512], DT, tag="rse", bufs=2, name=u.nm("rse"))
                nc.vector.reciprocal(rse[:], oacc2[hh][64:65, :])
                bcp = ps.tile([128, 512], FP, tag="bcp", bufs=1, name=u.nm("bcp"))
                nc.tensor.matmul(bcp[0:64, :], ones_row[0:1, 0:64],
                                 rse[:], start=True, stop=True)
                bcs = sb.tile([64, 512], DT, tag="bcs", bufs=2, name=u.nm("bcs"))
                nc.vector.tensor_copy(bcs[:], bcp[0:64, :])
                nc.vector.tensor_tensor(oTn[g][hs, sl], oacc2[hh][0:64, :],
                                        bcs[:], op=OP.mult)
        esG.close()
    with tc.tile_pool(name=u.nm("wops"), bufs=1, space="PSUM") as ps:
        for doc in range(8):
            wt = wsb.tile([128, 512], DT, tag="wo", bufs=2, name=u.nm("wo"))
            nc.sync.dma_start(wt[:], V(W["wo_packed"][li, doc]))
            for tb in range(NT):
                sl = slice(512 * tb, 512 * (tb + 1))
                acc = ps.tile([128, 512], FP, tag="woacc", bufs=3, name=u.nm("woa"))
                for dc in range(4):
                    nc.tensor.matmul(acc[:], wt[:, 128 * dc:128 * (dc + 1)],
                                     oTn[dc][:, sl], start=(dc == 0),
                                     stop=False)
                nc.tensor.matmul(acc[:], bor[0:1, 128 * doc:128 * (doc + 1)],
                                 ones_row[0:1, 0:512], start=False,
                                 stop=True)
                ob = sb.tile([128, 512], DT, tag="ob", bufs=3, name=u.nm("ob"))
                nc.scalar.copy(ob[:], acc[:])
                nc.sync.dma_start(V(ar1[128 * doc:128 * (doc + 1), sl]), ob[:])
            if doc == 3:
                nc.gpsimd.collective_compute(
                    "AllReduce", OP.add, replica_groups=RG,
                    ins=[ar1[0:512, :]], outs=[ar1o[0:512, :]])
    esW.close()
    nc.gpsimd.collective_compute("AllReduce", OP.add, replica_groups=RG,
                                 ins=[ar1[512:1024, :]], outs=[ar1o[512:1024, :]])
    xa = [xap.tile([128, Tl], DT, tag=f"xa{i}", bufs=1, name=u.nm("xa"))
          for i in range(8)]
    emit_ln(nc, tc, u, x_tiles, ar1o[:, :], W["ln1g_col"][li], W["ln1b_col"][li],
            C, Tl, None, dram, DT=DT, V=V, out_tiles=xa)
    esA.close()

    ar2 = dram.tile([D, Tl], FP, name=u.nm("ar2i"))
    ar2o = dram.tile([D, Tl], FP, name=u.nm("ar2o"))
    esI = ExitStack()
    wsb = esI.enter_context(tc.tile_pool(name=u.nm("fw"), bufs=2))
    hp = esI.enter_context(tc.tile_pool(name=u.nm("fh"), bufs=1))
    ps = esI.enter_context(tc.tile_pool(name=u.nm("fps"), bufs=1, space="PSUM"))
    b1c = wsb.tile([128, 16], FP, tag="b1c", bufs=1, name=u.nm("b1c"))
    nc.sync.dma_start(b1c[:], W["b1_col"][li])
    b2r = wsb.tile([1, 1024], DT, tag="b2r", bufs=1, name=u.nm("b2r"))
    nc.sync.dma_start(b2r[:], V(W["b2_row"][li]))
    NT2 = Tl // 1024
    for tb2 in range(NT2):
        hT = [hp.tile([128, 1024], DT, tag=f"hT{i % 8}", bufs=2, name=u.nm("hT"))
              for i in range(16)]
        for fc in range(16):
            wt = wsb.tile([128, 1024], DT, tag="w1", bufs=3, name=u.nm("w1"))
            nc.sync.dma_start(wt[:], V(W["w1_packed"][li, fc]))
            for hb in range(2):
                sl = slice(1024 * tb2 + 512 * hb, 1024 * tb2 + 512 * (hb + 1))
                acc = ps.tile([128, 512], FP, tag="hacc", bufs=3, name=u.nm("ha"))
                for dc in range(8):
                    nc.tensor.matmul(acc[:], wt[:, 128 * dc:128 * (dc + 1)],
                                     xa[dc][:, sl], start=(dc == 0),
                                     stop=(dc == 7))
                nc.scalar.activation(hT[fc][:, 512 * hb:512 * (hb + 1)], acc[:],
                                     AF.Relu, bias=b1c[:, fc:fc + 1])
        for doc in range(8):
            wt = wsb.tile([128, 2048], DT, tag="w2", bufs=3, name=u.nm("w2"))
            nc.sync.dma_start(wt[:], V(W["w2_packed"][li, doc]))
            for hb in range(2):
                slo = slice(1024 * tb2 + 512 * hb, 1024 * tb2 + 512 * (hb + 1))
                acc = ps.tile([128, 512], FP, tag="yacc", bufs=3, name=u.nm("ya"))
                for fc in range(16):
                    nc.tensor.matmul(acc[:], wt[:, 128 * fc:128 * (fc + 1)],
                                     hT[fc][:, 512 * hb:512 * (hb + 1)],
                                     start=(fc == 0), stop=False)
                nc.tensor.matmul(acc[:], b2r[0:1, 128 * doc:128 * (doc + 1)],
                                 ones_row[0:1, 0:512], start=False,
                                 stop=True)
                yb = wsb.tile([128, 512], DT, tag="yb", bufs=3, name=u.nm("yb"))
                nc.vector.tensor_copy(yb[:], acc[:])
                nc.sync.dma_start(V(ar2[128 * doc:128 * (doc + 1), slo]), yb[:])
            if doc == 3 and tb2 == NT2 - 1:
                nc.gpsimd.collective_compute(
                    "AllReduce", OP.add, replica_groups=RG,
                    ins=[ar2[0:512, :]], outs=[ar2o[0:512, :]])
    esI.close()
    nc.gpsimd.collective_compute("AllReduce", OP.add, replica_groups=RG,
                                 ins=[ar2[512:1024, :]], outs=[ar2o[512:1024, :]])
    emit_ln(nc, tc, u, xa, ar2o[:, :], W["ln2g_col"][li], W["ln2b_col"][li],
            C, Tl, out_dram, dram, DT=DT, V=V)
    esXA.close()


def emit_mod(nc, tc, u, li, x_dram, W, C, dram, out_dram, DT=FP, V=None):
    if V is None:
        V = lambda ap: ap
    xaug = dram.tile([T, 1088], FP, name=u.nm("xaug"))
    srow_d = dram.tile([1, T], FP, name=u.nm("srowd"))
    prow_d = dram.tile([1, T], FP, name=u.nm("prowd"))
    g_d = dram.tile([1, KSEL], I32, name=u.nm("gd"))
    w_d = dram.tile([1, KSEL], FP, name=u.nm("wdd"))
    xsel_d = dram.tile([D, KSEL], FP, name=u.nm("xseld"))
    proc_d = dram.tile([D, KSEL], FP, name=u.nm("procd"))
    gview = g_d[0:1, :].rearrange("a (b p) -> (a b) p", p=128).rearrange("b p -> p b")
    wview = w_d[0:1, :].rearrange("a (b p) -> (a b) p", p=128).rearrange("b p -> p b")
    # ---- routing + staging ----
    esA = ExitStack()
    xp = esA.enter_context(tc.tile_pool(name=u.nm("mxin"), bufs=1))
    x_tiles = load_x(nc, xp, u, x_dram, T)
    sb = esA.enter_context(tc.tile_pool(name=u.nm("msb"), bufs=2))
    rowp = esA.enter_context(tc.tile_pool(name=u.nm("mrow"), bufs=1))
    srow = rowp.tile([1, T], FP, tag="srow", bufs=1, name=u.nm("srow"))
    sP = sb.tile([128, 16], FP, tag="sP", bufs=1, name=u.nm("sP"))
    sbc = rowp.tile([128, T], FP, tag="sbc", bufs=1, name=u.nm("sbc"))
    with tc.tile_pool(name=u.nm("mp1"), bufs=1, space="PSUM") as ps:
        for tb in range(4):
            sl = slice(512 * tb, 512 * (tb + 1))
            acc = ps.tile([1, 512], FP, tag="sacc", bufs=2, name=u.nm("sa"))
            for dc in range(8):
                nc.tensor.matmul(acc[:], W["rw_col"][li][:, dc:dc + 1],
                                 x_tiles[dc][:, sl], start=(dc == 0), stop=(dc == 7))
            nc.vector.tensor_copy(srow[0:1, sl], acc[:])
        nc.sync.dma_start(srow_d[0:1, :], srow[:])
        s16 = sb.tile([16, 128], FP, tag="s16", bufs=1, name=u.nm("s16"))
        nc.sync.dma_start(s16[:],
                          srow_d[0:1, :].rearrange("a (b c) -> (a b) c", c=128))
        spp = ps.tile([128, 16], FP, tag="spp", bufs=1, name=u.nm("spp"))
        nc.tensor.transpose(spp[:], s16[:], C["ident"][0:16, 0:16])
        nc.vector.tensor_copy(sP[:], spp[:])
        for tb in range(4):
            sl = slice(512 * tb, 512 * (tb + 1))
            bp = ps.tile([128, 512], FP, tag="bp", bufs=2, name=u.nm("bp"))
            nc.tensor.matmul(bp[:], C["ones_row"][0:1, 0:128], srow[0:1, sl],
                             start=True, stop=True)
            nc.vector.tensor_copy(sbc[:, sl], bp[:])
    rank = rowp.tile([1, T], FP, tag="rank", bufs=1, name=u.nm("rank"))
    with tc.tile_pool(name=u.nm("mp2"), bufs=1, space="PSUM") as ps:
        racc = [ps.tile([1, 512], FP, tag=f"rk{i}", bufs=1, name=u.nm("rk"))
                for i in range(4)]
        for tci in range(16):
            # 0/1 indicators are exact in fp32r -> bit-exact counts at 1 cyc/row
            A = rowp.tile([128, T], FPR, tag="Acmp", bufs=2, name=u.nm("A"))
            with nc.allow_low_precision(reason="exact 0/1 rank counts"):
                nc.vector.tensor_scalar(A[:], sbc[:], sP[:, tci:tci + 1], None,
                                        OP.is_lt)
            for tb in range(4):
                nc.tensor.matmul(racc[tb][:], C["ones_col_r"][:, 0:1],
                                 A[:, 512 * tb:512 * (tb + 1)],
                                 start=(tci == 0), stop=(tci == 15))
        for tb in range(4):
            nc.vector.tensor_copy(rank[0:1, 512 * tb:512 * (tb + 1)], racc[tb][:])
    if os.environ.get("KDEBUG") and li == 1:
        dbg_rank = nc.dram_tensor("dbg_rank", [1, T], FP, kind="ExternalOutput")
        nc.sync.dma_start(dbg_rank[0:1, :], rank[:])
        dbg_srow = nc.dram_tensor("dbg_srow", [1, T], FP, kind="ExternalOutput")
        nc.sync.dma_start(dbg_srow[0:1, :], srow[:])
    mask = rowp.tile([1, T], FP, tag="mask", bufs=1, name=u.nm("mask"))
    nc.vector.tensor_scalar(mask[:], rank[:], float(KSEL) - 0.5, None, OP.is_lt)
    zr = rowp.tile([1, T], FP, tag="zr", bufs=1, name=u.nm("zr"))
    nc.vector.memset(zr[:], 0.0)
    pos = rowp.tile([1, T], FP, tag="pos", bufs=1, name=u.nm("pos"))
    nc.vector.tensor_tensor_scan(pos[:], mask[:], zr[:], 0.0, OP.add, OP.add)
    nc.vector.tensor_tensor(pos[:], pos[:], mask[:], op=OP.mult)
    nc.sync.dma_start(prow_d[0:1, :], pos[:])
    with tc.tile_pool(name=u.nm("mp3"), bufs=1, space="PSUM") as ps:
        p16 = sb.tile([16, 128], FP, tag="p16", bufs=1, name=u.nm("p16"))
        nc.sync.dma_start(p16[:],
                          prow_d[0:1, :].rearrange("a (b c) -> (a b) c", c=128))
        ppp = ps.tile([128, 16], FP, tag="ppp", bufs=1, name=u.nm("ppp"))
        nc.tensor.transpose(ppp[:], p16[:], C["ident"][0:16, 0:16])
        posP = sb.tile([128, 16], FP, tag="posP", bufs=1, name=u.nm("posP"))
        nc.vector.tensor_copy(posP[:], ppp[:])
        gacc = [ps.tile([1, 512], FP, tag=f"ga{i}", bufs=1, name=u.nm("ga"))
                for i in range(2)]
        for tci in range(16):
            # 0/1 match indicators and integer token ids (<=2047) are exact
            # in fp32r's 12-bit mantissa -> bit-exact gather indices
            R2 = rowp.tile([128, KSEL], FPR, tag="R2", bufs=2, name=u.nm("R2"))
            with nc.allow_low_precision(reason="exact 0/1 gather select"):
                nc.vector.tensor_scalar(R2[:], C["j1bc"][:, 0:KSEL],
                                        posP[:, tci:tci + 1], None, OP.is_equal)
            for gb in range(2):
                nc.tensor.matmul(gacc[gb][:], C["tokid_r"][:, tci:tci + 1],
                                 R2[:, 512 * gb:512 * (gb + 1)],
                                 start=(tci == 0), stop=(tci == 15))
        grow = sb.tile([1, KSEL], FP, tag="grow", bufs=1, name=u.nm("grow"))
        for gb in range(2):
            nc.vector.tensor_copy(grow[0:1, 512 * gb:512 * (gb + 1)], gacc[gb][:])
        gi = sb.tile([1, KSEL], I32, tag="gi", bufs=1, name=u.nm("gi"))
        nc.vector.tensor_copy(gi[:], grow[:])
        nc.sync.dma_start(g_d[0:1, :], gi[:])
        if os.environ.get("KDEBUG") and li == 1:
            dbg_g = nc.dram_tensor("dbg_g", [1, KSEL], FP, kind="ExternalOutput")
            nc.sync.dma_start(dbg_g[0:1, :], grow[:])
            dbg_pos = nc.dram_tensor("dbg_pos", [1, T], FP, kind="ExternalOutput")
            nc.sync.dma_start(dbg_pos[0:1, :], pos[:])
    with tc.tile_pool(name=u.nm("mp4"), bufs=1, space="PSUM") as ps:
        for tci in range(16):
            xn = sb.tile([128, 1088], FP, tag="xn", bufs=3, name=u.nm("xn"))
            for dc in range(8):
                tp = ps.tile([128, 128], FP, tag="tp", bufs=4, name=u.nm("tp"))
                nc.tensor.transpose(tp[:], x_tiles[dc][:, 128 * tci:128 * (tci + 1)],
                                    C["ident"][:])
                if dc % 2 == 0:
                    nc.vector.tensor_copy(xn[:, 128 * dc:128 * (dc + 1)], tp[:])
                else:
                    nc.scalar.copy(xn[:, 128 * dc:128 * (dc + 1)], tp[:])
            nc.vector.tensor_copy(xn[:, 1024:1025], sP[:, tci:tci + 1])
            nc.sync.dma_start(xaug[128 * tci:128 * (tci + 1), :], xn[:])
    esA.close()
    # ---- gather selected ----
    with tc.tile_pool(name=u.nm("gsb"), bufs=3) as sb2, \
         tc.tile_pool(name=u.nm("gxs"), bufs=1) as xsp, \
         tc.tile_pool(name=u.nm("gps2"), bufs=1, space="PSUM") as ps:
        xsel = [xsp.tile([128, KSEL], FP, tag=f"sel{i}", bufs=1, name=u.nm("xsel"))
                for i in range(8)]
        wP = sb2.tile([128, 8], FP, tag="wP", bufs=1, name=u.nm("wP"))
        gP = sb2.tile([128, 8], I32, tag="gP2", bufs=1, name=u.nm("gP2"))
        nc.sync.dma_start(gP[:], gview)
        for jc in range(8):
            xg = sb2.tile([128, 1088], FP, tag="xg", bufs=3, name=u.nm("xg"))
            nc.gpsimd.indirect_dma_start(
                xg[:], None, xaug[:, :],
                bass.IndirectOffsetOnAxis(ap=gP[:, jc:jc + 1], axis=0),
                bounds_check=T - 1, oob_is_err=False)
            for dc in range(8):
                tp = ps.tile([128, 128], FP, tag="tp2", bufs=4, name=u.nm("tp2"))
                nc.tensor.transpose(tp[:], xg[:, 128 * dc:128 * (dc + 1)],
                                    C["ident"][:])
                if dc % 2 == 0:
                    nc.vector.tensor_copy(xsel[dc][:, 128 * jc:128 * (jc + 1)], tp[:])
                else:
                    nc.scalar.copy(xsel[dc][:, 128 * jc:128 * (jc + 1)], tp[:])
            nc.scalar.activation(wP[:, jc:jc + 1], xg[:, 1024:1025], AF.Sigmoid)
        wtp = ps.tile([8, 128], FP, tag="wtp", bufs=1, name=u.nm("wtp"))
        nc.tensor.transpose(wtp[:], wP[:], C["ident"][:])
        wts = sb2.tile([8, 128], FP, tag="wts", bufs=1, name=u.nm("wts"))
        nc.vector.tensor_copy(wts[:], wtp[:])
        nc.sync.dma_start(w_d[0:1, :].rearrange("a (b c) -> (a b) c", c=128), wts[:])
        for dc in range(8):
            nc.sync.dma_start(xsel_d[128 * dc:128 * (dc + 1), :], xsel[dc][:])
        if os.environ.get("KDEBUG") and li == 1:
            dbg_xsel = nc.dram_tensor("dbg_xsel", [D, KSEL], FP, kind="ExternalOutput")
            for dc in range(8):
                nc.sync.dma_start(dbg_xsel[128 * dc:128 * (dc + 1), :], xsel[dc][:])
            dbg_w = nc.dram_tensor("dbg_w", [128, 8], FP, kind="ExternalOutput")
            nc.sync.dma_start(dbg_w[:, :], wP[:])
    # ---- encoder on selected ----
    emit_encoder(nc, tc, u, li, KSEL, xsel_d[:, :], W, C, dram, proc_d[:, :],
                 DT=DT, V=V)
    # ---- delta, scatter, rebuild ----
    with tc.tile_pool(name=u.nm("dsb"), bufs=3) as sb3, \
         tc.tile_pool(name=u.nm("dxp"), bufs=1) as dxp, \
         tc.tile_pool(name=u.nm("dps"), bufs=1, space="PSUM") as ps:
        wrow = sb3.tile([1, KSEL], FP, tag="wrow", bufs=1, name=u.nm("wrow"))
        nc.sync.dma_start(wrow[:], w_d[0:1, :])
        gP = sb3.tile([128, 8], I32, tag="gP3", bufs=1, name=u.nm("gP3"))
        nc.sync.dma_start(gP[:], gview)
        wbc = []
        for gb in range(2):
            bp = ps.tile([128, 512], FP, tag="wbp", bufs=2, name=u.nm("wbp"))
            nc.tensor.matmul(bp[:], C["ones_row"][0:1, 0:128],
                             wrow[0:1, 512 * gb:512 * (gb + 1)], start=True, stop=True)
            wb = sb3.tile([128, 512], FP, tag="wbc", bufs=2, name=u.nm("wbc"))
            nc.vector.tensor_copy(wb[:], bp[:])
            wbc.append(wb)
        for dc in range(8):
            xs = dxp.tile([128, KSEL], FP, tag="xs2", bufs=2, name=u.nm("xs2"))
            nc.sync.dma_start(xs[:], xsel_d[128 * dc:128 * (dc + 1), :])
            pr = dxp.tile([128, KSEL], FP, tag="pr2", bufs=2, name=u.nm("pr2"))
            nc.sync.dma_start(pr[:], proc_d[128 * dc:128 * (dc + 1), :])
            if os.environ.get("KDEBUG") and li == 1:
                if dc == 0 and not hasattr(nc, "_dbg_proc"):
                    nc._dbg_proc = nc.dram_tensor("dbg_proc", [D, KSEL], FP,
                                                  kind="ExternalOutput")
                nc.sync.dma_start(nc._dbg_proc[128 * dc:128 * (dc + 1), :], pr[:])
            ns = dxp.tile([128, KSEL], FP, tag="ns2", bufs=2, name=u.nm("ns2"))
            for gb in range(2):
                sl = slice(512 * gb, 512 * (gb + 1))
                d1 = sb3.tile([128, 512], FP, tag="d1", bufs=2, name=u.nm("d1"))
                nc.vector.tensor_tensor(d1[:], pr[:, sl], xs[:, sl], op=OP.subtract)
                nc.vector.tensor_tensor(d1[:], d1[:], wbc[gb][:], op=OP.mult)
                nc.vector.tensor_tensor(ns[:, sl], d1[:], xs[:, sl], op=OP.add)
            nc.sync.dma_start(proc_d[128 * dc:128 * (dc + 1), :], ns[:])
        for jc in range(8):
            nsl = []
            for dc in range(8):
                t = sb3.tile([128, 128], FP, tag=f"nsl{dc % 4}", bufs=3,
                             name=u.nm("nsl"))
                nc.sync.dma_start(t[:],
                                  proc_d[128 * dc:128 * (dc + 1),
                                         128 * jc:128 * (jc + 1)])
                nsl.append(t)
            nn_ = sb3.tile([128, 1088], FP, tag="nn", bufs=2, name=u.nm("nn"))
            nc.vector.memset(nn_[:, 1024:1088], 0.0)
            for dc in range(8):
                tp = ps.tile([128, 128], FP, tag="tp3", bufs=3, name=u.nm("tp3"))
                nc.tensor.transpose(tp[:], nsl[dc][:], C["ident"][:])
                if dc % 2 == 0:
                    nc.vector.tensor_copy(nn_[:, 128 * dc:128 * (dc + 1)], tp[:])
                else:
                    nc.scalar.copy(nn_[:, 128 * dc:128 * (dc + 1)], tp[:])
            nc.gpsimd.indirect_dma_start(
                xaug[:, :],
                bass.IndirectOffsetOnAxis(ap=gP[:, jc:jc + 1], axis=0),
                nn_[:], None, bounds_check=T - 1, oob_is_err=False)
        if os.environ.get("KDEBUG") and li == 1:
            dbg_xaug = nc.dram_tensor("dbg_xaug", [T, 1024], FP, kind="ExternalOutput")
            for tci in range(16):
                xga = sb3.tile([128, 1024], FP, tag="xga", bufs=2, name=u.nm("xga"))
                nc.sync.dma_start(xga[:], xaug[128 * tci:128 * (tci + 1), 0:1024])
                nc.sync.dma_start(dbg_xaug[128 * tci:128 * (tci + 1), :], xga[:])
        for tci in range(16):
            xr = sb3.tile([128, 1024], FP, tag="xrl", bufs=3, name=u.nm("xrl"))
            nc.sync.dma_start(xr[:], xaug[128 * tci:128 * (tci + 1), 0:1024])
            xo = sb3.tile([128, 1024], FP, tag="xo", bufs=3, name=u.nm("xo"))
            for dc in range(8):
                tp = ps.tile([128, 128], FP, tag="tp4", bufs=3, name=u.nm("tp4"))
                nc.tensor.transpose(tp[:], xr[:, 128 * dc:128 * (dc + 1)],
                                    C["ident"][:])
                if dc % 2 == 0:
                    nc.vector.tensor_copy(xo[:, 128 * dc:128 * (dc + 1)], tp[:])
                else:
                    nc.scalar.copy(xo[:, 128 * dc:128 * (dc + 1)], tp[:])
            for dc in range(8):
                nc.sync.dma_start(
                    out_dram[128 * dc:128 * (dc + 1), 128 * tci:128 * (tci + 1)],
                    xo[:, 128 * dc:128 * (dc + 1)])
    return


def build_nc():
    u = Ctr()
    nc = bacc.Bacc("TRN2", target_bir_lowering=False, debug=False, num_devices=8)
    Wd = {}
    Wd["wqkv_packed"] = nc.dram_tensor("wqkv_packed", [NL, 8, 128, 1024], FP,
                                       kind="ExternalInput")
    Wd["wv_rows"] = nc.dram_tensor("wv_rows", [NL, 8, 128, 512], FP,
                                   kind="ExternalInput")
    Wd["wo_packed"] = nc.dram_tensor("wo_packed", [NL, 8, 128, 512], FP,
                                     kind="ExternalInput")
    Wd["w1_packed"] = nc.dram_tensor("w1_packed", [NL, 16, 128, 1024], FP,
                                     kind="ExternalInput")
    Wd["w2_packed"] = nc.dram_tensor("w2_packed", [NL, 8, 128, 2048], FP,
                                     kind="ExternalInput")
    Wd["bqkv_row"] = nc.dram_tensor("bqkv_row", [NL, 1, 1024], FP,
                                    kind="ExternalInput")
    Wd["bv_row"] = nc.dram_tensor("bv_row", [NL, 1, 512], FP, kind="ExternalInput")
    Wd["bo_row"] = nc.dram_tensor("bo_row", [NL, 1, 1024], FP, kind="ExternalInput")
    Wd["b1_col"] = nc.dram_tensor("b1_col", [NL, 128, 16], FP, kind="ExternalInput")
    Wd["b2_row"] = nc.dram_tensor("b2_row", [NL, 1, 1024], FP, kind="ExternalInput")
    for nm in ("ln1g_col", "ln1b_col", "ln2g_col", "ln2b_col", "rw_col"):
        Wd[nm] = nc.dram_tensor(nm, [NL, 128, 8], FP, kind="ExternalInput")
    xT_d = nc.dram_tensor("xT", [D, T], FP, kind="ExternalInput")
    ident_d = nc.dram_tensor("ident", [128, 128], FP, kind="ExternalInput")
    j1bc_d = nc.dram_tensor("j1bc", [128, KSEL], FP, kind="ExternalInput")
    tokid_d = nc.dram_tensor("tokid", [128, 16], FP, kind="ExternalInput")
    out_d = nc.dram_tensor("out_xT", [D, T], FP, kind="ExternalOutput")

    class DramIdx:
        def __init__(self, ap):
            self.ap = ap

        def __getitem__(self, key):
            if isinstance(key, tuple):
                return self.ap[key[0], key[1]]
            return self.ap[key]

    with tile.TileContext(nc) as tc, ExitStack() as ctx:
        cpool = ctx.enter_context(tc.tile_pool(name="consts", bufs=1))
        dram = ctx.enter_context(tc.tile_pool(name="dram", bufs=1, space="DRAM"))
        C = {}
        C["ident"] = cpool.tile([128, 128], FP, tag="ident", bufs=1, name="identc")
        nc.sync.dma_start(C["ident"][:], ident_d[:, :])
        C["ones_row"] = cpool.tile([1, 512], FP, tag="onesr", bufs=1, name="onesr")
        nc.vector.memset(C["ones_row"][:], 1.0)
        C["ones_col"] = cpool.tile([128, 1], FP, tag="onesc", bufs=1, name="onesc")
        nc.vector.memset(C["ones_col"][:], 1.0)
        C["ones_row_r"] = cpool.tile([1, 512], FPR, tag="onesrr", bufs=1,
                                     name="onesrr")
        nc.vector.tensor_copy(C["ones_row_r"][:], C["ones_row"][:])
        C["ones_col_r"] = cpool.tile([128, 1], FPR, tag="onescr", bufs=1,
                                     name="onescr")
        nc.vector.tensor_copy(C["ones_col_r"][:], C["ones_col"][:])
        C["j1bc"] = cpool.tile([128, KSEL], FP, tag="j1bc", bufs=1, name="j1bc")
        nc.sync.dma_start(C["j1bc"][:], j1bc_d[:, :])
        C["tokid"] = cpool.tile([128, 16], FP, tag="tokid", bufs=1, name="tokid")
        nc.sync.dma_start(C["tokid"][:], tokid_d[:, :])
        C["tokid_r"] = cpool.tile([128, 16], FPR, tag="tokidr", bufs=1,
                                  name="tokidr")
        nc.vector.tensor_copy(C["tokid_r"][:], C["tokid"][:])

        W = {}
        for nm in ("wqkv_packed", "wv_rows", "wo_packed", "w1_packed",
                   "w2_packed"):
            W[nm] = DramIdx(Wd[nm])
        for nm in ("bqkv_row", "bv_row", "bo_row", "b2_row", "b1_col"):
            W[nm] = DramIdx(Wd[nm])
        for nm in ("ln1g_col", "ln1b_col", "ln2g_col", "ln2b_col", "rw_col"):
            tiles = []
            for li in range(NL):
                t = cpool.tile([128, 8], FP, tag=f"{nm}{li}", bufs=1,
                               name=f"{nm}{li}")
                nc.sync.dma_start(t[:], Wd[nm][li])
                tiles.append(t)
            W[nm] = tiles

        xd = [dram.tile([D, T], FP, name=f"xd{i}") for i in range(NL + 1)]
        with tc.tile_pool(name="x0p", bufs=1) as x0p:
            for dc in range(8):
                t = x0p.tile([128, T], FP, tag=f"x0{dc}", bufs=1, name=f"x0_{dc}")
                nc.sync.dma_start(t[:], xT_d[128 * dc:128 * (dc + 1), :])
                nc.sync.dma_start(xd[0][128 * dc:128 * (dc + 1), :], t[:])
        nlayers = int(os.environ.get("KLAYERS", NL))
        modes = os.environ.get("KMODES", "ffffff")
        for li in range(nlayers):
            DTl, Vl = mkdv(modes[li])
            lp = (nc.allow_low_precision(reason="fp32r layer")
                  if DTl is FPR else nullcontext())
            with lp:
                if li % 2 == 1:
                    emit_mod(nc, tc, u, li, xd[li][:, :], W, C, dram,
                             xd[li + 1][:, :], DT=DTl, V=Vl)
                else:
                    emit_encoder(nc, tc, u, li, T, xd[li][:, :], W, C, dram,
                                 xd[li + 1][:, :], DT=DTl, V=Vl)
        with tc.tile_pool(name="xfp", bufs=1) as xfp:
            for dc in range(8):
                t = xfp.tile([128, T], FP, tag=f"xf{dc}", bufs=1, name=f"xf_{dc}")
                nc.sync.dma_start(t[:], xd[nlayers][128 * dc:128 * (dc + 1), :])
                nc.sync.dma_start(out_d[128 * dc:128 * (dc + 1), :], t[:])
    nc.compile()
    return nc


def _pack_inputs(x, Wqkv, bqkv, Wo, bo, W1, b1, W2, b2,
                 ln1g, ln1b, ln2g, ln2b, router_w):
    f32 = np.float32
    maps = []
    ident = np.eye(128, dtype=f32)
    j1bc = np.broadcast_to(np.arange(1, KSEL + 1, dtype=f32), (128, KSEL)).copy()
    tokid = (np.arange(16)[None, :] * 128 + np.arange(128)[:, None]).astype(f32)
    lncols = {
        "ln1g_col": ln1g.reshape(NL, 8, 128).transpose(0, 2, 1).astype(f32).copy(),
        "ln1b_col": ln1b.reshape(NL, 8, 128).transpose(0, 2, 1).astype(f32).copy(),
        "ln2g_col": ln2g.reshape(NL, 8, 128).transpose(0, 2, 1).astype(f32).copy(),
        "ln2b_col": ln2b.reshape(NL, 8, 128).transpose(0, 2, 1).astype(f32).copy(),
        "rw_col": router_w.reshape(NL, 8, 128).transpose(0, 2, 1).astype(f32).copy(),
    }
    for c in range(8):
        p, h = c // 2, c % 2
        fs = slice(DFH * h, DFH * (h + 1))
        m = {"xT": np.ascontiguousarray(x[p].T)}
        wq = np.empty((NL, 8, 128, 1024), f32)
        wvr = np.empty((NL, 8, 128, 512), f32)
        wop = np.empty((NL, 8, 128, 512), f32)
        w1p = np.empty((NL, 16, 128, 1024), f32)
        w2p = np.empty((NL, 8, 128, 2048), f32)
        bqr = np.empty((NL, 1, 1024), f32)
        bvr = np.empty((NL, 1, 512), f32)
        bor = np.empty((NL, 1, 1024), f32)
        b1c = np.empty((NL, 128, 16), f32)
        b2r = np.empty((NL, 1, 1024), f32)
        for l in range(NL):
            Wq = Wqkv[l][512 * h:512 * (h + 1)].T
            Wk = Wqkv[l][D + 512 * h:D + 512 * (h + 1)].T
            Wv = Wqkv[l][2 * D + 512 * h:2 * D + 512 * (h + 1)].T
            qkcat = np.concatenate([Wq, Wk], axis=1)
            for cc in range(8):
                blk = qkcat[:, 128 * cc:128 * (cc + 1)]
                wq[l, cc] = blk.reshape(8, 128, 128).transpose(1, 0, 2).reshape(128, 1024)
            for dc in range(8):
                wvr[l, dc] = Wv[128 * dc:128 * (dc + 1), :]
            WoT_s = Wo[l].T[512 * h:512 * (h + 1), :]
            for doc in range(8):
                blk = WoT_s[:, 128 * doc:128 * (doc + 1)]
                wop[l, doc] = blk.reshape(4, 128, 128).transpose(1, 0, 2).reshape(128, 512)
            W1T_s = W1[l][fs].T
            for fc in range(16):
                blk = W1T_s[:, 128 * fc:128 * (fc + 1)]
                w1p[l, fc] = blk.reshape(8, 128, 128).transpose(1, 0, 2).reshape(128, 1024)
            W2T_s = W2[l].T[fs, :]
            for doc in range(8):
                blk = W2T_s[:, 128 * doc:128 * (doc + 1)]
                w2p[l, doc] = blk.reshape(16, 128, 128).transpose(1, 0, 2).reshape(128, 2048)
            bqr[l, 0] = np.concatenate([bqkv[l][:D][512 * h:512 * (h + 1)],
                                        bqkv[l][D:2 * D][512 * h:512 * (h + 1)]])
            bvr[l, 0] = bqkv[l][2 * D:][512 * h:512 * (h + 1)]
            bor[l, 0] = bo[l] * 0.5
            b1c[l] = b1[l][fs].reshape(16, 128).T
            b2r[l, 0] = b2[l] * 0.5
        m.update(wqkv_packed=wq, wv_rows=wvr, wo_packed=wop, w1_packed=w1p,
                 w2_packed=w2p, bqkv_row=bqr, bv_row=bvr, bo_row=bor,
                 b1_col=b1c, b2_row=b2r, ident=ident, j1bc=j1bc, tokid=tokid)
        m.update(lncols)
        maps.append(m)
    return maps


def kernel(**inputs):
    inputs = {k: np.asarray(v, dtype=np.float32) for k, v in inputs.items()}
    if "nc" not in _CACHED:
        _CACHED["nc"] = build_nc()
    nc = _CACHED["nc"]
    maps = _pack_inputs(**inputs)
    kw = {}
    if os.environ.get("KTRACE"):
        kw["trace"] = True
        kw["tmpdir"] = os.environ.get("KTRACE_DIR") or None
    res = bass_utils.run_bass_kernel_spmd(nc, maps, core_ids=list(range(8)), **kw)
    _CACHED["last_res"] = res
    out = np.empty((B, T, D), np.float32)
    for p in range(B):
        out[p] = res.results[2 * p]["out_xT"].T
    return out

